# revision 7
# baseline (speedup 1.0000x reference)
"""Trainium2 Bass kernel for nn_ConditionalSplineSQ2D.

Math:
  out[b] = sum_{g,h,c} coeffs[g,h,c] * p[b,g,h,ii_c] * p[b,g,h,jj_c]
         = sum_{cells} p_cell^T S_cell p_cell            (S_cell symmetric 8x8)
         = sum_{cells} sum_k lam[cell,k] * (V[cell]^T p_cell)_k^2

Host precomputes the eigendecomposition of the 961 8x8 matrices; the device
kernel per 16-cell group does:
  mm1 (PE, 4x col-tiled): T = Wblk^T @ P    (block-diag stationary, fp16)
  sq  (ACT or DVE):       Q = T*T           (PSUM -> SBUF fp16, 3 groups/op)
  mm2 (PE, 4x col-tiled): acc[32j,:] += lam_g^T @ Q_g  (4 groups concurrent)

Sharding: pure data parallel over batch (512 per core x 8 cores); the
4 partial accumulator rows per core are summed on host.
"""

import numpy as np

B, G, P = 4096, 31, 8
NCORES = 8
NB = B // NCORES  # 512 batches per core
CELLS = G * G  # 961
GROUP_CELLS = 16
NGROUPS = -(-CELLS // GROUP_CELLS)  # 61
CELLS_PAD = NGROUPS * GROUP_CELLS  # 976
PARTS = 128
CH = 8   # groups per DMA chunk
TG = 3   # groups per PSUM tile (3 banks)
NT = -(-NGROUPS // TG)  # 21 psum tiles
ACT_TILE_STRIDE = 5  # of every 5 psum tiles, 2 go to DVE (rest ACT)

_nc_cache = {}


def _build_nc():
    import concourse.mybir as mybir
    import concourse.tile as tile
    from concourse import bacc

    nc = bacc.Bacc()
    pt_d = nc.dram_tensor(
        "pt", [PARTS, NGROUPS * NB], mybir.dt.float16, kind="ExternalInput"
    )
    w_d = nc.dram_tensor(
        "wblk", [PARTS, NGROUPS * PARTS], mybir.dt.float16, kind="ExternalInput"
    )
    lam_d = nc.dram_tensor(
        "lamt", [PARTS, NGROUPS], mybir.dt.float16, kind="ExternalInput"
    )
    out_d = nc.dram_tensor("out", [4, NB], mybir.dt.float32, kind="ExternalOutput")

    nchunks = -(-NGROUPS // CH)

    with tile.TileContext(nc) as tc:
        with (
            tc.tile_pool(name="const", bufs=1) as cpool,
            tc.tile_pool(name="ptp", bufs=3) as ppool,
            tc.tile_pool(name="qp", bufs=3) as qpool,
            tc.tile_pool(name="psp", bufs=2, space="PSUM") as pspool,
            tc.tile_pool(name="accp", bufs=1, space="PSUM") as apool,
        ):
            w_sb = cpool.tile([PARTS, NGROUPS * PARTS], mybir.dt.float16)
            lam_sb = cpool.tile([PARTS, NGROUPS], mybir.dt.float16)
            nc.sync.dma_start(out=lam_sb[:, :], in_=lam_d[:, :])
            acc = apool.tile([PARTS, NB], mybir.dt.float32)

            # stream param + weights per chunk of CH groups
            ptiles = []
            for ci in range(nchunks):
                g0 = ci * CH
                ch = min(CH, NGROUPS - g0)
                nc.sync.dma_start(
                    out=w_sb[:, g0 * PARTS : (g0 + ch) * PARTS],
                    in_=w_d[:, g0 * PARTS : (g0 + ch) * PARTS],
                )
                ptile = ppool.tile([PARTS, CH * NB], mybir.dt.float16, tag="ptile")
                nc.sync.dma_start(
                    out=ptile[:, : ch * NB],
                    in_=pt_d[:, g0 * NB : (g0 + ch) * NB],
                )
                ptiles.append(ptile)

            q_slices = {}  # group -> (q_tile, slot)
            n_rounds = -(-NGROUPS // 4)  # 16 mm2 rounds of up to 4 groups
            rounds_emitted = 0
            last_round_of_pos = {}  # col pos j -> last round index using it
            for r in range(n_rounds):
                for j in range(4):
                    if r * 4 + j < NGROUPS:
                        last_round_of_pos[j] = r

            def emit_mm2_rounds(limit_group):
                """Emit mm2 rounds whose groups are all squared (< limit)."""
                nonlocal rounds_emitted
                while rounds_emitted < n_rounds:
                    r = rounds_emitted
                    hi = min(r * 4 + 4, NGROUPS)
                    if hi > limit_group:
                        return
                    for j in range(4):
                        g = r * 4 + j
                        if g >= NGROUPS:
                            break
                        qt, slot = q_slices.pop(g)
                        nc.tensor.matmul(
                            acc[32 * j : 32 * j + 1, :],
                            lam_sb[:, g : g + 1],
                            qt[:, slot * NB : (slot + 1) * NB],
                            start=(r == 0),
                            stop=(r == last_round_of_pos[j]),
                            tile_position=(0, 32 * j),
                        )
                    rounds_emitted += 1

            for t in range(NT):
                tg0 = t * TG
                ng = min(TG, NGROUPS - tg0)
                psT = pspool.tile([PARTS, TG * NB], mybir.dt.float32, tag="psT")
                for gg in range(ng):
                    g = tg0 + gg
                    ci, off = divmod(g, CH)
                    # mm1: 4 concurrent 128x32 column tiles
                    for j in range(4):
                        nc.tensor.matmul(
                            psT[32 * j : 32 * j + 32, gg * NB : (gg + 1) * NB],
                            w_sb[:, g * PARTS + 32 * j : g * PARTS + 32 * j + 32],
                            ptiles[ci][:, off * NB : (off + 1) * NB],
                            start=True,
                            stop=True,
                            tile_position=(0, 32 * j),
                        )
                q = qpool.tile([PARTS, TG * NB], mybir.dt.float16, tag="q")
                if t % ACT_TILE_STRIDE < ACT_TILE_STRIDE - 2:
                    # ScalarE: square straight out of PSUM (one big op)
                    nc.scalar.square(q[:, : ng * NB], psT[:, : ng * NB])
                else:
                    # VectorE: copy-cast to SBUF fp16, then square at 2x
                    qc = qpool.tile([PARTS, TG * NB], mybir.dt.float16, tag="qc")
                    nc.vector.tensor_copy(qc[:, : ng * NB], psT[:, : ng * NB])
                    nc.vector.tensor_mul(
                        q[:, : ng * NB], qc[:, : ng * NB], qc[:, : ng * NB]
                    )
                for gg in range(ng):
                    q_slices[tg0 + gg] = (q, gg)
                # weight-reduce every fully-squared round of 4 groups, but
                # stay one tile behind so PE never waits on a fresh square
                emit_mm2_rounds(tg0)
            emit_mm2_rounds(NGROUPS)

            # evacuate the 4 accumulator rows (partition-aligned copies)
            out_sb = cpool.tile([PARTS, NB], mybir.dt.float32)
            for j in range(4):
                nc.vector.tensor_copy(
                    out_sb[32 * j : 32 * j + 1, :], acc[32 * j : 32 * j + 1, :]
                )
                nc.sync.dma_start(
                    out=out_d[j : j + 1, :], in_=out_sb[32 * j : 32 * j + 1, :]
                )
    if not nc.is_finalized():
        nc.finalize()
    return nc


def _get_nc():
    if "nc" not in _nc_cache:
        _nc_cache["nc"] = _build_nc()
    return _nc_cache["nc"]


def _host_prep_weights(integral_coeffs):
    """coeffs [G,G,C] -> (wblk [128, NGROUPS*128] fp16, lamt [128, NGROUPS] fp16)."""
    ii, jj = np.triu_indices(P)
    w = integral_coeffs.reshape(CELLS, len(ii)).astype(np.float64)
    S = np.zeros((CELLS, P, P), np.float64)
    # quadratic form: off-diag split in half, diag gets full coeff
    np.add.at(S, (slice(None), ii, jj), 0.5 * w)
    np.add.at(S, (slice(None), jj, ii), 0.5 * w)
    lam, V = np.linalg.eigh(S)  # V columns are eigenvectors

    lam_p = np.zeros((CELLS_PAD, P))
    lam_p[:CELLS] = lam
    V_p = np.zeros((CELLS_PAD, P, P))
    V_p[:CELLS] = V

    # block-diagonal stationary: wb[g, 8t+i, 8t+k] = V[16g+t, i, k]
    Vg = V_p.reshape(NGROUPS, GROUP_CELLS, P, P)
    wb = np.zeros((NGROUPS, GROUP_CELLS, P, GROUP_CELLS, P), np.float32)
    t = np.arange(GROUP_CELLS)
    wb[:, t, :, t, :] = Vg.transpose(1, 0, 2, 3)
    wblk = (
        wb.reshape(NGROUPS, PARTS, PARTS)
        .transpose(1, 0, 2)
        .reshape(PARTS, NGROUPS * PARTS)
        .astype(np.float16)
    )
    lamt = np.ascontiguousarray(
        lam_p.reshape(NGROUPS, PARTS).T.astype(np.float16)
    )
    return np.ascontiguousarray(wblk), lamt


def _host_prep_param(param_tensor):
    """param [B,G,G,P] f32 -> list of per-core [128, NGROUPS*NB] fp16 arrays."""
    flat = param_tensor.reshape(B, CELLS * P)
    out = []
    for c in range(NCORES):
        shard = flat[c * NB : (c + 1) * NB]
        pad = np.zeros((NB, CELLS_PAD * P), np.float32)
        pad[:, : CELLS * P] = shard
        # (b, g, p) -> (p, g, b)
        pt = (
            pad.reshape(NB, NGROUPS, PARTS)
            .transpose(2, 1, 0)
            .reshape(PARTS, NGROUPS * NB)
            .astype(np.float16)
        )
        out.append(np.ascontiguousarray(pt))
    return out


def _run(param_tensor, integral_coeffs, trace=False, **run_kwargs):
    from concourse.bass_utils import run_bass_kernel_spmd

    nc = _get_nc()
    wblk, lamt = _host_prep_weights(np.asarray(integral_coeffs, np.float32))
    pts = _host_prep_param(np.asarray(param_tensor, np.float32))
    in_maps = [{"pt": pts[c], "wblk": wblk, "lamt": lamt} for c in range(NCORES)]
    res = run_bass_kernel_spmd(
        nc, in_maps, core_ids=list(range(NCORES)), trace=trace, **run_kwargs
    )
    out = np.concatenate(
        [res.results[c]["out"].sum(axis=0).reshape(NB) for c in range(NCORES)]
    ).astype(np.float32)
    return out, res


def kernel(param_tensor, integral_coeffs):
    out, _ = _run(param_tensor, integral_coeffs)
    return out
